# revision 15
# baseline (speedup 1.0000x reference)
"""Int-infer matmul kernel v3 for trn2, 8 NeuronCores, data-parallel over (b,h).

reference: y = clip(round(matmul(clip(round(x1*r1)), clip(round(x2*r2))) / 16), -128, 127)
shapes: x1 [2,16,2048,64] f32, x2 [2,16,64,2048] f32 -> y [2,16,2048,2048] f32

v3 = v2's evict/ring structure with PE transposes (the xbar DMA-transpose is
~70x slower on real silicon than the cost model's 14ns/tile).

Per core (4 of the 32 (b,h) pairs, 2 supersteps of 2 pairs):
 - host prep: x1i = clip(round(x1*r1)) bf16, pre-swizzled to [128, 16*64]
   per pair (partition p = s%128, chunk c = s//128) so the load is a plain
   contiguous partition split; x2s = clip(round(x2*r2))/16 bf16 packed
   [128, 2048] per superstep (pair A rows 0:64, pair B 64:128). All values
   exact in bf16 (ints in [-128,127]; /16 = exponent shift). Folding /16
   into x2 makes the evict a pure f32->i8 convert.
 - prologue: PE-transpose all x1 chunks ([128,64] -> psum [64,128], pair A to
   partitions 0:64 / B to 64:128 via tile_position), copy psum->SBUF x1T
   (DVE 2x_1p bf16 / ACT, deficit-split). The tp pool (1 PSUM bank x2) closes
   before the main ring opens, so the ring still gets all 8 banks.
 - mains: K=64 row-packed matmuls via tile_position (0,0)/(64,0); PSUM is one
   [128, 4096] f32 ring: per m-tile, pair A fills [0:2048] (4x N=512), pair B
   fills [2048:4096]. Evicts = 4 staggered [128,1024] spans per m-round
   (PSUM f32 -> SBUF i8, RNE+saturate == clip(round(.))), deficit-balanced
   across ACT and DVE. 4 spans keep the PE refills off the evict critical
   path (fewer/bigger spans pipeline worse, measured in CoreSim).
 - output: i8 staging ring [128, 8192] (2 m-rounds), one DMA per pair per 2
   m-tiles (256KB, 2KB/descriptor); final round drains per-m to shrink the
   tail. Output upcast to f32 on host.
"""
import sys

sys.path.insert(0, "/opt/trn_rl_repo")

import numpy as np
import ml_dtypes
import concourse.bass as bass
import concourse.bacc as bacc
import concourse.mybir as mybir
import concourse.tile as tile
from concourse.bass_utils import run_bass_kernel_spmd
from concourse.masks import make_identity

F32 = mybir.dt.float32
BF16 = mybir.dt.bfloat16
I8 = mybir.dt.int8
AF = mybir.ActivationFunctionType

N_CORES = 8
PAIRS_PER_CORE = 4
S = 2048
D = 64
N_MM = 512    # moving free dim per matmul (one PSUM bank)
RING = 2 * S  # full PSUM: 4096 f32 per partition
NSPAN = 4
N_CHUNK = S // 128  # 16 s-chunks of 128 rows per pair


def build_program(repeat: int = 1, nspan=NSPAN) -> bass.Bass:
    nc = bacc.Bacc("TRN2", target_bir_lowering=False, debug=False, num_devices=N_CORES)
    n_ss = PAIRS_PER_CORE // 2
    # x1s[ss]: [128, 2048] bf16, pair-interleaved swizzle:
    # (p, c*128 + d)      = x1i[pair 2ss  ][c*128+p, d]
    # (p, c*128 + 64 + d) = x1i[pair 2ss+1][c*128+p, d]
    # so ONE [128,128] PE transpose per chunk yields both pairs packed at
    # partitions 0:64 / 64:128 - half the transpose instructions
    x1s = nc.dram_tensor("x1s", [PAIRS_PER_CORE // 2, 128, N_CHUNK * 2 * D], BF16,
                         kind="ExternalInput").ap()
    x2p = nc.dram_tensor("x2p", [n_ss, 2 * D, S], BF16, kind="ExternalInput").ap()
    y = nc.dram_tensor("y", [PAIRS_PER_CORE, S, S], I8, kind="ExternalOutput").ap()

    if repeat > 1:
        nc.dram_tensor("rep_marker", [1, repeat], F32, kind="ExternalInput")

    if isinstance(nspan, (list, tuple)):
        bounds = [0]
        for sz in nspan:
            bounds.append(bounds[-1] + sz)
        assert bounds[-1] == RING
    else:
        bounds = [round(i * RING / nspan) for i in range(nspan + 1)]
    spans = list(zip(bounds[:-1], bounds[1:]))

    with tile.TileContext(nc) as tc:
      for _rep in range(repeat):
        ev = {"act": 0.0, "dve": 0.0}

        def assign(cost_act, cost_dve):
            if ev["act"] + cost_act <= ev["dve"] + cost_dve:
                ev["act"] += cost_act
                return "act"
            ev["dve"] += cost_dve
            return "dve"

        with (
            tc.tile_pool(name="x1raw", bufs=4) as x1raw_pool,
            tc.tile_pool(name="x1T", bufs=2) as x1T_pool,
            tc.tile_pool(name="x2t", bufs=2) as x2t_pool,
            tc.tile_pool(name="const", bufs=1) as const_pool,
        ):
            identity = const_pool.tile([128, 128], BF16)
            make_identity(nc, identity)

            x1Ts = []
            x2ts = []
            # prologue: load + PE-transpose all pairs' x1 into SBUF x1T tiles.
            # The dummy pool pins the tp tiles to PSUM banks 6-7 so the main
            # ring's early banks (cols 0:3072) carry no WAR against the
            # prologue - only mm0's last B-windows wait on the prologue tail.
            with tc.tile_pool(name="dummy", bufs=1, space="PSUM") as dummy_pool, \
                 tc.tile_pool(name="tpsum", bufs=2, space="PSUM") as tpsum_pool:
                dummy_pool.tile([128, 3072], F32, tag="dummy", name="dummy")
                raws = {}
                # x1 raws first (transposes gate everything); ss0 chunked so
                # the first transpose group starts early
                for ss in range(n_ss):
                    raws[ss] = x1raw_pool.tile([128, N_CHUNK * 2 * D], BF16,
                                               tag="x1raw", name=f"x1raw{ss}")
                q = N_CHUNK * 2 * D // 4
                for ci in range(4):
                    nc.sync.dma_start(out=raws[0][:, ci * q:(ci + 1) * q],
                                      in_=x1s[0, :, ci * q:(ci + 1) * q])
                nc.sync.dma_start(out=raws[1][:], in_=x1s[1])
                for ss in range(n_ss):
                    x2t = x2t_pool.tile([128, S], BF16, tag="x2t", name=f"x2t{ss}")
                    nc.sync.dma_start(out=x2t[:, 0:S // 2], in_=x2p[ss, :, 0:S // 2])
                    nc.sync.dma_start(out=x2t[:, S // 2:S], in_=x2p[ss, :, S // 2:S])
                    x2ts.append(x2t)
                for ss in range(n_ss):
                    x1T = x1T_pool.tile([128, S], BF16, tag="x1T", name=f"x1T{ss}")
                    for g in range(N_CHUNK // 4):  # 4 tp tiles per superstep
                        tp = tpsum_pool.tile([128, 4 * 128], BF16, tag="tp",
                                             name=f"tp{ss}_{g}")
                        for j in range(4):
                            c = g * 4 + j
                            nc.tensor.transpose(
                                tp[:, j * 128:(j + 1) * 128],
                                raws[ss][:, c * 128:(c + 1) * 128],
                                identity[:],
                            )
                        dst = x1T[:, g * 512:(g + 1) * 512]
                        # psum bf16 copy: DVE gets 2x_1p (2-byte packed)
                        if assign(1056.0, 694.0) == "act":
                            nc.scalar.activation(dst, tp[:], AF.Copy)
                        else:
                            nc.vector.tensor_copy(dst, tp[:])
                    x1Ts.append(x1T)

            with tc.tile_pool(name="ost", bufs=4) as ost_pool, \
                 tc.tile_pool(name="mpsum", bufs=1, space="PSUM") as mpsum_pool:
                ring = mpsum_pool.tile([128, RING], F32, tag="ring")

                for ss in range(n_ss):
                    x1T, x2t = x1Ts[ss], x2ts[ss]
                    for mq in range(8):  # 8 rounds of 2 m-tiles
                        last_round = ss == n_ss - 1 and mq == 7
                        ost = ost_pool.tile([128, 2 * RING], I8, tag="ost")
                        for mm in range(2):
                            m = 2 * mq + mm
                            mcols = slice(m * 128, (m + 1) * 128)
                            for half in (0, 1):
                                lo, hi = half * 64, half * 64 + 64
                                for w in range(S // N_MM):
                                    cols = slice(half * S + w * N_MM,
                                                 half * S + (w + 1) * N_MM)
                                    nc.tensor.matmul(
                                        ring[:, cols],
                                        lhsT=x1T[lo:hi, mcols],
                                        rhs=x2t[lo:hi, w * N_MM:(w + 1) * N_MM],
                                        start=True,
                                        stop=True,
                                        tile_position=(half * 64, 0),
                                    )
                            cur_spans = spans
                            if ss == 0 and mq == 0 and mm == 0:
                                cur_spans = [(i * 512, (i + 1) * 512) for i in range(8)]
                            for (a, b) in cur_spans:
                                dst = ost[:, mm * RING + a:mm * RING + b]
                                fd = b - a
                                if assign(1.02 * ((172 + fd) / 1.2 + 59),
                                          (120 + fd) / 0.96 + 36) == "act":
                                    nc.scalar.activation(dst, ring[:, a:b], AF.Copy)
                                else:
                                    nc.vector.tensor_copy(dst, ring[:, a:b])
                            if last_round:
                                for half in (0, 1):
                                    p = 2 * ss + half
                                    nc.sync.dma_start(
                                        out=y[p].rearrange("(m pp) c -> pp m c", pp=128)[
                                            :, 2 * mq + mm:2 * mq + mm + 1, :
                                        ],
                                        in_=ost.rearrange("p (m h c) -> p (m h) c",
                                                          m=2, h=2)[
                                            :, 2 * mm + half:2 * mm + half + 1, :
                                        ],
                                    )
                        if not last_round:
                            for half in (0, 1):
                                p = 2 * ss + half
                                nc.sync.dma_start(
                                    out=y[p].rearrange("(m pp) c -> pp m c", pp=128)[
                                        :, 2 * mq:2 * mq + 2, :
                                    ],
                                    in_=ost.rearrange("p (m h c) -> p (m h) c",
                                                      m=2, h=2)[:, half::2, :],
                                )

    nc.compile()
    return nc


_CACHE: dict = {}


def _prep(x1, x2, r1, r2):
    """Host-side quantizer rescale + layout packing (all values bf16-exact)."""
    x1i = np.clip(np.round(x1 * np.float32(r1)), -128.0, 127.0)
    x2s = np.clip(np.round(x2 * np.float32(r2)), -128.0, 127.0) * np.float32(1.0 / 16.0)
    n = x1.shape[0]
    # pair-interleaved swizzle: [pairs, S, D] -> per ss [128, (c, pair, d)]
    x1v = x1i.reshape(n // 2, 2, N_CHUNK, 128, D).transpose(0, 3, 2, 1, 4)
    x1v = x1v.reshape(n // 2, 128, N_CHUNK * 2 * D)
    x2p = x2s.reshape(n // 2, 2 * D, S)
    return (
        np.ascontiguousarray(x1v).astype(ml_dtypes.bfloat16),
        np.ascontiguousarray(x2p).astype(ml_dtypes.bfloat16),
    )


def kernel(x1, x2, scale1_last_layer, scale_x1, scale2_last_layer, scale_x2):
    x1 = np.asarray(x1, dtype=np.float32)
    x2 = np.asarray(x2, dtype=np.float32)
    r1 = float(np.float32(scale1_last_layer) / np.float32(scale_x1))
    r2 = float(np.float32(scale2_last_layer) / np.float32(scale_x2))

    if "nc" not in _CACHE:
        _CACHE["nc"] = build_program()
    nc = _CACHE["nc"]

    b, h = x1.shape[0], x1.shape[1]
    x1r = x1.reshape(b * h, S, D)
    x2r = x2.reshape(b * h, D, S)
    in_maps = []
    for c in range(N_CORES):
        sl = slice(c * PAIRS_PER_CORE, (c + 1) * PAIRS_PER_CORE)
        x1s_, x2p_ = _prep(x1r[sl], x2r[sl], r1, r2)
        in_maps.append({"x1s": x1s_, "x2p": x2p_})
    res = run_bass_kernel_spmd(nc, in_maps, list(range(N_CORES)))
    out = np.concatenate([r["y"] for r in res.results], axis=0)
    return out.reshape(b, h, S, S).astype(np.float32)


if __name__ == "__main__":
    rng = np.random.default_rng(0)
    x1 = np.round(np.clip(rng.normal(size=(2, 16, S, D)) * 40.0, -128, 127)).astype(np.float32)
    x2 = np.round(np.clip(rng.normal(size=(2, 16, D, S)) * 40.0, -128, 127)).astype(np.float32)
    y = kernel(x1, x2, np.float32(0.1), np.float32(0.05), np.float32(0.08), np.float32(0.04))
    print("out", y.shape, y.dtype, y[0, 0, :2, :8])
